# revision 14
# baseline (speedup 1.0000x reference)
"""Trainium2 Bass kernel for nn_Complex_Only_46308337385506 (gnn_message_passing).

Math (same reduction as validated v1 kernel):
  Per-edge basis R enters only via two scalars:
      gam = Jz/|J|, rho = sqrt(Jx^2+Jy^2)/|J|, a1 = copysign(rho, gam)
  With w = gam*Xz - a1*Xx:
      Y0 = Wa@Xx + (Wa-Wc)@(a1*w) + Wb@(gam*Xy)
      Y1 = Wa@Xy - Wb@(a1*Xz + gam*Xx)
      Y2 = Wa@Xz + (Wc-Wa)@(gam*w) + Wb@(a1*Xy)
  VN leaky-relu: d = Wd@Y, out = Y - 0.8*min(<Y,d>,0)/(|d|^2+eps)*d
  (d computed scaled by 2^-6; the correction is scale-invariant in d)

v2 speedups (cost-model driven):
  - fp16 on-device; inputs/outputs converted + laid out on HOST (planar comp,
    partition-major): input DMA halved, output DMA halved.
  - DVE: custom 2-in ops for sum-of-squares, TensorTensor in 2x fp16 mode,
    tensor_scalar in 4x mode; 2-tensor scalar_tensor_tensor avoided (no perf
    modes).
  - all matmuls/transposes fp16 -> 1 cycle/row on PE (vs 4 for f32).
  - rsqrt/reciprocal via Ln/Exp on ACT (single pinned table).
  - work spread across DVE/Pool/ACT/SP(dma) queues.

Sharding: one batch per core (B=8).
"""

import math
import numpy as np
from contextlib import ExitStack

import concourse.bass as bass
import concourse.bacc as bacc
import concourse.tile as tile
from concourse import mybir
from concourse import bass_utils

F32 = mybir.dt.float32
F16 = mybir.dt.float16
U16 = mybir.dt.uint16
AF = mybir.ActivationFunctionType
ALU = mybir.AluOpType

B, C, E = 8, 16384, 64
SUPER = 1024           # points per super-iteration
NSUP = C // SUPER      # 16
GROUP = 512            # matmul free dim (points)
NCHUNK = 8             # 128-pt chunks per super

LN08 = float(math.log(0.8))

_CUSTOM_OPS = {}


def _register_custom_dve_ops():
    """Register two fused DVE ops (module-level, idempotent):
      SQSUM_ANT: out = Src0^2 + Src1^2
      ADDSQ_ANT: out = Src0 + Src1^2"""
    if _CUSTOM_OPS:
        return _CUSTOM_OPS
    import numpy as _np
    from concourse import dve_ops
    from concourse.dve_spec import Spec, Src0, Src1, lower, sq, _has_src1
    from concourse.dve_uop import DveOpSpec
    from concourse.dve_table_gen import dve_ver_for

    def make(name, body, ref):
        spec = Spec(body=body, reference=ref)
        opcode = dve_ops._CUSTOM_DVE_ROW_BASE + len(dve_ops.OPS)
        shas = {}
        for ver in ("v3", "v4"):
            try:
                s = DveOpSpec(name=name, opcode=opcode,
                              uops=lower(spec, ver=ver),
                              rd1_en=_has_src1(spec))
                shas[ver] = s.sha(ver)
            except Exception:
                pass
        op = dve_ops.DveOp(name, spec, subdim=False, uops_sha=shas)
        dve_ops.OPS.append(op)
        dve_ops.CUSTOM_DVE_SPECS[name] = spec
        dve_ops._SUB_OPCODE_FOR_NAME[name] = opcode
        assert opcode < 0x20
        return op

    def _r2(a):
        return a.astype(_np.float32).reshape(a.shape[0], -1)

    _CUSTOM_OPS["SQSUM"] = make(
        "SQSUM_ANT", sq(Src0) + sq(Src1),
        lambda in0, in1, s0, s1, imm2: _r2(in0) ** 2 + _r2(in1) ** 2)
    _CUSTOM_OPS["ADDSQ"] = make(
        "ADDSQ_ANT", Src0 + sq(Src1),
        lambda in0, in1, s0, s1, imm2: _r2(in0) + _r2(in1) ** 2)
    return _CUSTOM_OPS


def _pin_act_table_set(arch: str):
    """Keep only natural_log_exp_and_others serving Ln/Exp/Copy so exactly
    one ACT table load is emitted."""
    from concourse import hw_specs
    tables = hw_specs.get_activation_tables(arch)
    mine = {AF.Ln, AF.Exp, AF.Copy, AF.Identity}
    for name, funcs in tables.items():
        if name != "natural_log_exp_and_others":
            funcs -= mine


def _build_nc():
    global OPS
    OPS = _register_custom_dve_ops()
    nc = bacc.Bacc("TRN2", debug=False)
    _pin_act_table_set(nc.m.arch)

    # host-prepped planar fp16 layouts (see _prep_in below)
    XH = nc.dram_tensor("XH", [128, NSUP, NCHUNK * 192], F16,
                        kind="ExternalInput").ap()
    JH = nc.dram_tensor("JH", [128, NSUP, NCHUNK * 192], F16,
                        kind="ExternalInput").ap()
    WMM = nc.dram_tensor("WMM", [6, 128, 128], F16, kind="ExternalInput").ap()
    OUT = nc.dram_tensor("OUT", [128, NSUP, 3 * GROUP], F16,
                         kind="ExternalOutput").ap()

    with tile.TileContext(nc) as tc, ExitStack() as ctx:
        const = ctx.enter_context(tc.tile_pool(name="const", bufs=1))
        io = ctx.enter_context(tc.tile_pool(name="io", bufs=2))
        sa = ctx.enter_context(tc.tile_pool(name="sa", bufs=2))
        pr = ctx.enter_context(tc.tile_pool(name="pr", bufs=2))
        rhp = ctx.enter_context(tc.tile_pool(name="rhp", bufs=2))
        xsp = ctx.enter_context(tc.tile_pool(name="xsp", bufs=2))
        s3p = ctx.enter_context(tc.tile_pool(name="s3p", bufs=2))
        outp = ctx.enter_context(tc.tile_pool(name="outp", bufs=2))
        psT = ctx.enter_context(tc.tile_pool(name="psT", bufs=3, space="PSUM"))
        psY = ctx.enter_context(tc.tile_pool(name="psY", bufs=2, space="PSUM"))
        psD = ctx.enter_context(tc.tile_pool(name="psD", bufs=1, space="PSUM"))

        # bias constants for ACT + sign mask
        eps6_c = const.tile([128, 1], F32, tag="eps6_c")
        eps12_c = const.tile([128, 1], F32, tag="eps12_c")
        eps4_c = const.tile([128, 1], F32, tag="eps4_c")
        ln08_c = const.tile([128, 1], F32, tag="ln08_c")
        sgn16_c = const.tile([128, 1], U16, tag="sgn16_c")
        nc.gpsimd.memset(eps6_c[:], 1e-6)
        nc.gpsimd.memset(eps12_c[:], 1e-12)
        nc.gpsimd.memset(eps4_c[:], 1e-4)
        nc.gpsimd.memset(ln08_c[:], LN08)
        nc.gpsimd.memset(sgn16_c[:], 0x8000)

        # weights + identity (fp16), loaded once
        wsb = const.tile([128, 6, 128], F16)
        nc.sync.dma_start(wsb[:], WMM.rearrange("n p m -> p n m"))
        LW_A = wsb[:, 0, :]      # blkdiag(WaT, WaT)
        LW_2 = wsb[:, 1, :]      # blkdiag((Wa-Wc).T, (Wc-Wa).T)
        LW_B = wsb[:, 2, :]      # blkdiag(WbT, WbT)
        LW_1 = wsb[:, 3, 0:64]   # [WaT; -WbT], M=64
        LW_D = wsb[:, 4, :]      # blkdiag(WdT, WdT) * 2^-6
        IDT = wsb[:, 5, :]       # identity (fp16)

        def v3(t):  # [128, 512] tile -> [128, 8, 64] view
            return t[:].rearrange("p (s e) -> p s e", s=NCHUNK, e=E)

        pending_store = [None]

        for u in range(NSUP):
            # ---- input loads: pairs of supers, fp16 planar --------------
            # (issued before the previous super's store so the in-order SP
            # queue never blocks loads behind the deep-pipeline store dep)
            if u % 2 == 0:
                xs2 = io.tile([128, 2, NCHUNK * 192], F16, tag="xs")
                js2 = io.tile([128, 2, NCHUNK * 192], F16, tag="js")
                nc.sync.dma_start(xs2[:], XH[:, u:u + 2])
                nc.sync.dma_start(js2[:], JH[:, u:u + 2])
            if pending_store[0] is not None:
                uprev, otprev = pending_store[0]
                nc.sync.dma_start(OUT[:, uprev],
                                  otprev[:].rearrange("p a b -> p (a b)"))
                pending_store[0] = None
            xs = xs2[:, u % 2]
            js = js2[:, u % 2]
            # planar views: [128, s, comp, e]
            xv = xs.rearrange("p (s c e) -> p s c e", s=NCHUNK, c=3, e=E)
            jv = js.rearrange("p (s c e) -> p s c e", s=NCHUNK, c=3, e=E)
            # host comp order is [x, z, y] so the (x,z) pair is contiguous
            jx, jz, jy = jv[:, :, 0, :], jv[:, :, 1, :], jv[:, :, 2, :]
            xx, xz, xy = xv[:, :, 0, :], xv[:, :, 1, :], xv[:, :, 2, :]

            # ---- stage A: per-edge scalars gam, a1 (fp16) ---------------
            q = sa.tile([128, SUPER // 2], F16, tag="q")
            n2 = sa.tile([128, SUPER // 2], F16, tag="n2")
            nc.vector._custom_dve(OPS["SQSUM"], out=v3(q), in0=jx, in1=jy)
            nc.vector._custom_dve(OPS["ADDSQ"], out=v3(n2), in0=v3(q), in1=jz)

            ln2 = sa.tile([128, SUPER // 2], F16, tag="ln2")
            lq = sa.tile([128, SUPER // 2], F16, tag="lq")
            nc.scalar.activation(ln2[:], n2[:], AF.Ln, bias=eps6_c[:])
            nc.scalar.activation(lq[:], q[:], AF.Ln, bias=eps12_c[:])
            t_ = sa.tile([128, SUPER // 2], F16, tag="t_")
            nc.scalar.activation(t_[:], ln2[:], AF.Exp, scale=-0.5)
            df = sa.tile([128, SUPER // 2], F16, tag="df")
            nc.gpsimd.tensor_tensor(df[:], lq[:], ln2[:], ALU.subtract)
            rho = sa.tile([128, SUPER // 2], F16, tag="rho")
            nc.scalar.activation(rho[:], df[:], AF.Exp, scale=0.5)
            gam = sa.tile([128, SUPER // 2], F16, tag="gam")
            nc.gpsimd.tensor_tensor(v3(gam), jz, v3(t_), ALU.mult)
            sg = sa.tile([128, SUPER // 2], F16, tag="sg")
            a1 = sa.tile([128, SUPER // 2], F16, tag="a1")
            nc.vector.tensor_scalar(sg[:].bitcast(U16), gam[:].bitcast(U16),
                                    0x8000, None, ALU.bitwise_and)
            nc.vector.tensor_tensor(a1[:].bitcast(U16), sg[:].bitcast(U16),
                                    rho[:].bitcast(U16), ALU.bitwise_or)

            # ---- stage B: products into pair tiles (fp16) ---------------
            P23 = pr.tile([128, NCHUNK, 2, E], F16, tag="P23")  # [a1*w; gam*w]
            P45 = pr.tile([128, NCHUNK, 2, E], F16, tag="P45")  # [gam*xy; a1*xy]
            PY7 = pr.tile([128, NCHUNK, 2, E], F16, tag="PY7")  # [xy; s7]
            m1 = sa.tile([128, SUPER // 2], F16, tag="m1")
            m2 = sa.tile([128, SUPER // 2], F16, tag="m2")
            m3 = sa.tile([128, SUPER // 2], F16, tag="m3")
            m4 = sa.tile([128, SUPER // 2], F16, tag="m4")
            wt = sa.tile([128, SUPER // 2], F16, tag="wt")
            nc.gpsimd.tensor_tensor(v3(m1), v3(gam), xz, ALU.mult)
            nc.gpsimd.tensor_tensor(v3(m2), v3(a1), xx, ALU.mult)
            nc.gpsimd.tensor_tensor(wt[:], m1[:], m2[:], ALU.subtract)
            nc.gpsimd.tensor_tensor(P23[:, :, 0, :], v3(a1), v3(wt), ALU.mult)
            nc.vector.tensor_tensor(P23[:, :, 1, :], v3(gam), v3(wt), ALU.mult)
            nc.gpsimd.tensor_tensor(P45[:, :, 0, :], v3(gam), xy, ALU.mult)
            nc.gpsimd.tensor_tensor(P45[:, :, 1, :], v3(a1), xy, ALU.mult)
            nc.gpsimd.tensor_tensor(v3(m3), v3(a1), xz, ALU.mult)
            nc.gpsimd.tensor_tensor(v3(m4), v3(gam), xx, ALU.mult)
            # ycopy via SP-queue DMA (SBUF->SBUF), frees DVE/Pool
            nc.sync.dma_start(PY7[:, :, 0, :], xv[:, :, 2, :])
            nc.gpsimd.tensor_tensor(PY7[:, :, 1, :], v3(m3), v3(m4), ALU.add)

            # ---- per group: transpose (fp16), rh copies, matmuls --------
            xsb = xsp.tile([128, 3, GROUP], F16, tag="xsb")
            for g in range(2):
                rh = rhp.tile([128, 4, GROUP], F16, tag="rh")
                for h in range(2):          # half-groups of 2 chunks
                    tp = psT.tile([128, 4, 256], F16, tag="tp")
                    for k in range(2):
                        s = 4 * g + 2 * h + k
                        sl = slice(128 * k, 128 * (k + 1))
                        nc.tensor.transpose(
                            tp[:, 0, sl], xv[:, s, 0:2, :], IDT)
                        nc.tensor.transpose(
                            tp[:, 1, sl], P23[:, s, :, :], IDT)
                        nc.tensor.transpose(
                            tp[:, 2, sl], P45[:, s, :, :], IDT)
                        nc.tensor.transpose(
                            tp[:, 3, sl], PY7[:, s, :, :], IDT)
                    ho = slice(256 * h, 256 * (h + 1))
                    # one merged strided copy per half-group
                    nc.vector.tensor_copy(rh[:, :, ho], tp[:])

                py = psY.tile([128, 2, GROUP], F32, tag="py")
                pA = py[:, 0, :]
                pB = py[0:64, 1, :]
                nc.tensor.matmul(pA, LW_A, rh[:, 0, :], start=True, stop=False)
                nc.tensor.matmul(pA, LW_2, rh[:, 1, :], start=False,
                                 stop=False)
                nc.tensor.matmul(pA, LW_B, rh[:, 2, :], start=False, stop=True)
                nc.tensor.matmul(pB, LW_1, rh[:, 3, :], start=True, stop=True)

                ro = slice(64 * g, 64 * (g + 1))
                # Y0,Y1 in one copy (ACT); Y2 on Pool
                nc.scalar.activation(
                    xsb[ro, 0:2, :].rearrange("p a b -> p (a b)"),
                    py[0:64, :, :].rearrange("p a b -> p (a b)"), AF.Copy)
                nc.scalar.activation(xsb[ro, 2, :], py[64:128, 0, :],
                                     AF.Copy)

            # ---- Wd stage + VN leaky relu (fp16, d scaled by 2^-6) ------
            dh = s3p.tile([128, 3, GROUP], F16, tag="dh")
            for i in range(3):
                pd = psD.tile([128, GROUP], F32, tag="pd")
                nc.tensor.matmul(pd[:], LW_D, xsb[:, i, :], start=True,
                                 stop=True)
                if i == 2:
                    nc.vector.tensor_scalar(dh[:, i, :], pd[:], 2.0 ** -6,
                                            None, ALU.mult)
                else:
                    nc.scalar.activation(dh[:, i, :], pd[:], AF.Copy,
                                         scale=2.0 ** -6)

            d0, d1, d2 = dh[:, 0, :], dh[:, 1, :], dh[:, 2, :]
            dn2 = s3p.tile([128, GROUP], F16, tag="dn2")
            nc.vector._custom_dve(OPS["SQSUM"], out=dn2[:], in0=d0, in1=d1)
            nc.vector._custom_dve(OPS["ADDSQ"], out=dn2[:], in0=dn2[:],
                                  in1=d2)

            lnv = s3p.tile([128, GROUP], F16, tag="lnv")
            rec = s3p.tile([128, GROUP], F16, tag="rec")
            nc.scalar.activation(lnv[:], dn2[:], AF.Ln, bias=eps4_c[:])
            nc.scalar.activation(rec[:], lnv[:], AF.Exp, scale=-1.0,
                                 bias=ln08_c[:])

            xd = s3p.tile([128, 3, GROUP], F16, tag="xd")
            dot = s3p.tile([128, GROUP], F16, tag="dot")
            s2m = s3p.tile([128, GROUP], F16, tag="s2m")
            s2 = s3p.tile([128, GROUP], F16, tag="s2")
            nc.vector.tensor_tensor(
                xd[:].rearrange("p a b -> p (a b)"),
                xsb[:].rearrange("p a b -> p (a b)"),
                dh[:].rearrange("p a b -> p (a b)"), ALU.mult)
            nc.gpsimd.tensor_tensor(dot[:], xd[:, 0, :], xd[:, 1, :], ALU.add)
            nc.gpsimd.tensor_tensor(dot[:], dot[:], xd[:, 2, :], ALU.add)
            nc.gpsimd.tensor_scalar(s2m[:], dot[:], 0.0, None, ALU.min)
            nc.gpsimd.tensor_tensor(s2[:], s2m[:], rec[:], ALU.mult)

            ot = outp.tile([128, 3, GROUP], F16, tag="ot")
            mi = s3p.tile([128, 3, GROUP], F16, tag="mi")
            for i in range(3):
                nc.gpsimd.tensor_tensor(mi[:, i, :], s2[:], dh[:, i, :],
                                        ALU.mult)
            nc.vector.tensor_tensor(
                ot[:].rearrange("p a b -> p (a b)"),
                xsb[:].rearrange("p a b -> p (a b)"),
                mi[:].rearrange("p a b -> p (a b)"), ALU.subtract)

            pending_store[0] = (u, ot)

        uprev, otprev = pending_store[0]
        nc.sync.dma_start(OUT[:, uprev],
                          otprev[:].rearrange("p a b -> p (a b)"))

    nc.compile()
    return nc


_NC = None


def _get_nc():
    global _NC
    if _NC is None:
        _NC = _build_nc()
    return _NC


def _weight_stack(Wa, Wb, Wc, Wd):
    Z = np.zeros((64, 64), np.float32)

    def blk(a, b):
        return np.block([[a, Z], [Z, b]])

    WaT = np.asarray(Wa, np.float32).T
    WbT = np.asarray(Wb, np.float32).T
    WcT = np.asarray(Wc, np.float32).T
    WdT = np.asarray(Wd, np.float32).T
    w = np.stack([
        blk(WaT, WaT),
        blk(WaT - WcT, WcT - WaT),
        blk(WbT, WbT),
        np.block([[WaT, Z], [-WbT, Z]]),
        blk(WdT, WdT),
        np.eye(128, dtype=np.float32),
    ])
    return np.ascontiguousarray(w).astype(np.float16)


def _prep_in(A):
    """[C, E, 3] f32 -> [128, NSUP, NCHUNK*192] fp16 planar comp-major.
    point c = u*1024 + s*128 + p; row = s*192 + comp*64 + e."""
    v = A[..., [0, 2, 1]]                           # comp order x, z, y
    v = v.reshape(NSUP, NCHUNK, 128, E, 3)          # u, s, p, e, comp
    v = v.transpose(2, 0, 1, 4, 3)                  # p, u, s, comp, e
    return np.ascontiguousarray(v.astype(np.float16)
                                .reshape(128, NSUP, NCHUNK * 192))


def _post_out(o):
    """[128, NSUP, 3*GROUP] fp16 -> [64, 3, C] f32.
    partition p = g*64 + f; point c = u*1024 + g*512 + col."""
    v = o.reshape(2, 64, NSUP, 3, GROUP)            # g, f, u, i, col
    v = v.transpose(1, 3, 2, 0, 4)                  # f, i, u, g, col
    return np.ascontiguousarray(v).reshape(64, 3, C).astype(np.float32)


def run_full(X, J, Wa, Wb, Wc, Wd, trace=False, trace_kwargs=None):
    nc = _get_nc()
    wmm = _weight_stack(Wa, Wb, Wc, Wd)
    X = np.asarray(X, np.float32)
    J = np.asarray(J, np.float32)
    in_maps = []
    for b in range(B):
        in_maps.append({
            "XH": _prep_in(X[b]),
            "JH": _prep_in(J[b]),
            "WMM": wmm,
        })
    res = bass_utils.run_bass_kernel_spmd(
        nc, in_maps, core_ids=list(range(B)), trace=trace,
        **(trace_kwargs or {}))
    out = np.stack([_post_out(np.asarray(res.results[b]["OUT"]))
                    for b in range(B)])
    return out, res


def kernel(X, J, Wa, Wb, Wc, Wd):
    out, _ = run_full(X, J, Wa, Wb, Wc, Wd)
    return out
